# revision 50
# baseline (speedup 1.0000x reference)
"""BEV-pool (lift-splat-shoot scatter-sum) Trainium2 Bass kernel.

Pipeline
--------
Host (numpy, index math only):
  * mirror the reference geometry in float32 to voxelize every frustum
    point (value-identical to the jax/CPU reference on in-bounds points)
  * compact the ~2k occupied voxels into a dense slot space; split each
    voxel's points round-robin across the 8 cores so every core sees the
    SAME padded segment layout (slot s owns ceil(cnt_s/8) positions) —
    one shared SPMD program, per-core data
  * lay each core's points out slot-sorted and chunk-transposed in DRAM
    ([128, nch*80] bf16) so the device streams them with plain wide
    contiguous DMAs at full line rate (no indirect DMA, no SWDGE)
  * 128-point chunks are slot-sorted, so each chunk touches a <=32-wide
    slot window (rare wider spans just emit one extra matmul); bake the
    per-matmul window-relative slot ids into a small meta tensor

Device (per core, Bass/Tile):
  * the whole [80ch x 2044slot] accumulator grid lives in 4 PSUM banks;
    one start=True zero-matmul per bank initializes it
  * per matmul: one-hot rhs [128pts, 32slots] built on DVE by is_equal
    against an iota row (batched 16 matmuls per DVE instruction); points
    tile is the stationary lhsT so the matmul moves only 32 columns
  * PSUM -> SBUF stage copies on the Activation engine per bank, then
    plain DMAs write the [80, 2044] bf16 partial back

Host combine: sum the 8 aligned partials in fp32, scatter the compact
slot rows into the zeros output grid.

A post-pass splits multi-wait instructions into single-wait
EventSemaphores (this walrus build accepts only one sync-wait slot per
instruction struct).
"""

import os
import numpy as np
import ml_dtypes

BF16 = ml_dtypes.bfloat16

# ---- problem constants (from the reference nn.Module) ----
IMAGE_SIZE = (256, 704)
FEATURE_SIZE = (32, 88)
XBOUND = (-54.0, 54.0, 0.3)
YBOUND = (-54.0, 54.0, 0.3)
ZBOUND = (-10.0, 10.0, 20.0)
DBOUND = (1.0, 60.0, 1.0)

N_CORES = 8
P = 128          # points per chunk / matmul contraction dim
OH_W = 32        # one-hot window width (moving cols per matmul)
BANK_W = 512     # PSUM bank width in fp32
KB = 16          # matmuls per batched DVE is_equal
CS = 32          # chunks per stream slab DMA
NB = 16          # slab buffers in flight
NOB = 21         # one-hot buffers in flight


def _host_geometry(img_trans, img_scale, lidar2img, B, N, D, H, W):
    """float32 numpy mirror of the reference get_geometry + voxelize."""
    dx = np.array([XBOUND[2], YBOUND[2], ZBOUND[2]], np.float32)
    bx = np.array([XBOUND[0] + XBOUND[2] / 2.0,
                   YBOUND[0] + YBOUND[2] / 2.0,
                   ZBOUND[0] + ZBOUND[2] / 2.0], np.float32)
    nx = [int((b[1] - b[0]) / b[2]) for b in (XBOUND, YBOUND, ZBOUND)]
    NX, NY, NZ = nx

    iH, iW = IMAGE_SIZE
    fH, fW = FEATURE_SIZE
    ds = np.arange(DBOUND[0], DBOUND[1], DBOUND[2], dtype=np.float32)
    xs = np.linspace(0.0, iW - 1, fW, dtype=np.float32)
    ys = np.linspace(0.0, iH - 1, fH, dtype=np.float32)
    assert ds.shape[0] == D and fH == H and fW == W

    fr = np.stack([
        np.broadcast_to(xs[None, None, :], (D, H, W)),
        np.broadcast_to(ys[None, :, None], (D, H, W)),
        np.broadcast_to(ds[:, None, None], (D, H, W)),
    ], axis=-1).astype(np.float32)                       # [D,H,W,3]

    pts = fr[None, None] + img_trans[:, :, None, None, None, :]
    d = pts[..., 2:3]
    xy = pts[..., :2] / img_scale[:, :, None, None, None, None]
    p4 = np.concatenate([xy * d, d, np.ones_like(d)], axis=-1)
    img2lidar = np.linalg.inv(lidar2img)
    geom = np.einsum('bnij,bndhwj->bndhwi', img2lidar, p4)[..., :3]
    geom = geom.astype(np.float32)
    vox = ((geom - (bx - dx / 2.0)) / dx).astype(np.int32)  # trunc toward 0
    mask = ((vox[..., 0] >= 0) & (vox[..., 0] < NX)
            & (vox[..., 1] >= 0) & (vox[..., 1] < NY)
            & (vox[..., 2] >= 0) & (vox[..., 2] < NZ))
    flat = (vox[..., 2] * NX + vox[..., 0]) * NY + vox[..., 1]
    flat = flat + np.arange(B, dtype=np.int32)[:, None, None, None, None] \
        * (NZ * NX * NY)
    flatm = np.where(mask, flat, -1).reshape(-1)
    return flatm, (NX, NY, NZ)


def _build_schedule(slot_of_pos, nch, S):
    """Per-matmul windows over the shared slot-sorted point layout.

    Returns (mm list of (chunk, bank, col_lo, width), lids [P, n_mmp] f32,
    last_mm_of_bank {bank: mm index}).
    """
    mms = []
    lid_cols = []
    for j in range(nch):
        sl = slot_of_pos[j * P:(j + 1) * P]
        real = sl[sl >= 0]
        if len(real) == 0:
            continue
        cur, hi = int(real.min()), int(real.max())
        while True:
            bank = cur // BANK_W
            wend = min(cur + OH_W, (bank + 1) * BANK_W, S)
            w = wend - cur
            lid = np.where((sl >= cur) & (sl < wend), sl - cur, -1.0)
            mms.append((j, bank, cur - bank * BANK_W, w))
            lid_cols.append(lid.astype(np.float32))
            nxt = real[real >= wend]
            if len(nxt) == 0:
                break
            cur = int(nxt.min())
    n_mm = len(mms)
    n_mmp = -(-n_mm // KB) * KB
    lids = np.full((P, n_mmp), -1.0, np.float32)
    if n_mm:
        lids[:, :n_mm] = np.stack(lid_cols, axis=1)
    last_mm_of_bank = {}
    for i, (j, bank, lo, w) in enumerate(mms):
        last_mm_of_bank[bank] = i
    return mms, lids, n_mmp, last_mm_of_bank


def _wb_ranges(mms, S):
    """Writeback column ranges + the last matmul touching each; the ranges
    are written back the moment their last matmul retires."""
    RW = 256
    bounds = [0] + list(range(RW, S, RW)) + [S]
    ranges = list(zip(bounds[:-1], bounds[1:]))
    last_mm_of_rng = {}
    for i, (j, bank, lo, w) in enumerate(mms):
        c0, c1 = bank * BANK_W + lo, bank * BANK_W + lo + w
        for r, (r0, r1) in enumerate(ranges):
            if c0 < r1 and c1 > r0:
                last_mm_of_rng[r] = i
    by_completion = sorted(last_mm_of_rng, key=lambda r: last_mm_of_rng[r])
    return ranges, last_mm_of_rng, by_completion


def _slab_plan(nch):
    """Chunk counts per slab DMA: small head (fast pipeline fill), CS-sized
    middles, strictly shrinking tail (short post-stream dependency chain)."""
    tail = [c for c in (12, 8, 4) if nch > 48]
    mid = nch - sum(tail)
    plan = [CS] * (mid // CS)
    if mid % CS:
        plan.append(mid % CS)
    plan += tail
    assert sum(plan) == nch and plan
    return plan


def _build_bass(nch, n_mmp, mms, last_mm_of_bank, S, C,
                range_split=False, last_engs=("sync", "scalar")):
    import concourse.bass as bass
    import concourse.mybir as mybir
    import concourse.tile as tile

    f32 = mybir.dt.float32
    bf16 = mybir.dt.bfloat16
    n_banks = -(-S // BANK_W)
    # meta free layout: [iota: OH_W | lids: n_mmp]
    MW = OH_W + n_mmp
    plan = _slab_plan(nch)
    slab_of_chunk = np.repeat(np.arange(len(plan)), plan)
    slab_c0 = np.concatenate([[0], np.cumsum(plan)[:-1]])

    ranges, last_mm_of_rng, by_completion = _wb_ranges(mms, S)

    nc = bass.Bass()
    pts = nc.dram_tensor("pts", [P, nch * C], bf16, kind="ExternalInput")
    meta = nc.dram_tensor("meta", [P, MW], bf16, kind="ExternalInput")
    outp = nc.dram_tensor("out", [C, S], bf16, kind="ExternalOutput")

    with tile.TileContext(nc) as tc:
        with tc.tile_pool(name="sb", bufs=1) as con, \
             tc.tile_pool(name="ps", bufs=1, space="PSUM") as ps:
            meta_sb = con.tile([P, MW], bf16, tag="meta")
            scratch = con.tile([P, BANK_W], bf16, tag="scratch")
            stage = con.tile([P, S], bf16, tag="stage")
            slabs = [con.tile([P, int(w) * C], bf16, name=f"slab{k}",
                              tag=f"slab{k}") for k, w in enumerate(plan)]
            ohs = [con.tile([P, KB * OH_W], bf16, name=f"oh{k}",
                            tag=f"oh{k}") for k in range(-(-len(mms) // KB))]
            accs = [ps.tile([P, BANK_W], f32, name=f"acc{k}", tag=f"acc{k}")
                    for k in range(n_banks)]

            # zero-matmul feed with no DMA dependency: PSUM init starts
            # during the framework preamble
            nc.gpsimd.memset(scratch[:], 0.0)
            # first stream slab before meta: its transfer reaches the DMA
            # engines ~1.3us after issue (HWDGE + DGE latency), so the short
            # meta transfer slots in behind it instead of leaving the DMA
            # engines idle while slab0's issue chain drains
            nc.sync.dma_start(out=slabs[0][:],
                              in_=pts[:, 0:int(plan[0]) * C])
            nc.sync.dma_start(out=meta_sb[:], in_=meta[:])

            # one start=True matmul per PSUM bank zeroes the whole 2KB zero
            # region (0 x 0); afterwards arbitrary overlapping start=False
            # accumulation windows are legal
            for t in range(n_banks):
                nc.tensor.matmul(
                    out=accs[t][0:C, 0:BANK_W],
                    lhsT=scratch[:, 0:C],
                    rhs=scratch[:],
                    start=True, stop=False)

            for i, w in enumerate(plan):
                if i == 0:
                    continue
                c0 = int(slab_c0[i])
                nc.sync.dma_start(out=slabs[i][:],
                                  in_=pts[:, c0 * C:(c0 + int(w)) * C])

            mstride = meta_sb[:].ap[0][0]
            for m, (j, bank, lo, w) in enumerate(mms):
                b = m // KB
                if m % KB == 0:
                    oh = ohs[b]
                    out_ap = bass.AP(oh[:].tensor, 0,
                                     [[KB * OH_W, P], [OH_W, KB], [1, OH_W]])
                    iota_ap = bass.AP(meta_sb[:].tensor, 0,
                                      [[mstride, P], [0, KB], [1, OH_W]])
                    lid_ap = bass.AP(meta_sb[:].tensor, OH_W + KB * b,
                                     [[mstride, P], [1, KB], [0, OH_W]])
                    nc.vector.tensor_tensor(
                        out=out_ap, in0=iota_ap, in1=lid_ap,
                        op=mybir.AluOpType.is_equal)
                si = int(slab_of_chunk[j])
                cj = j - int(slab_c0[si])
                nc.tensor.matmul(
                    out=accs[bank][0:C, lo:lo + w],
                    lhsT=slabs[si][:, cj * C:(cj + 1) * C],
                    rhs=ohs[b][:, (m % KB) * OH_W:(m % KB) * OH_W + w],
                    start=False, stop=(m == last_mm_of_bank[bank]))
                for r, (r0, r1) in enumerate(ranges):
                    if last_mm_of_rng.get(r) != m:
                        continue
                    w2 = r1 - r0
                    bank_r, lo_r = r0 // BANK_W, r0 % BANK_W
                    nc.scalar.activation(
                        out=stage[0:C, r0:r0 + w2],
                        in_=accs[bank_r][0:C, lo_r:lo_r + w2],
                        func=mybir.ActivationFunctionType.Copy)
                    # mid-stream writebacks ride the Pool SWDGE path: its sem
                    # pool is separate from the HWDGE sems, so slab DMAs never
                    # stall on an out-DMA's completion through sem-slot reuse.
                    # The last two writebacks use the (by then idle) SP and
                    # Act HWDGE queues so they don't serialize on Pool's
                    # 1us-per-DMA descriptor generation.
                    if r == by_completion[-1] and last_engs[0]:
                        eng = getattr(nc, last_engs[0])
                    elif r == by_completion[-2] and last_engs[1]:
                        eng = getattr(nc, last_engs[1])
                    else:
                        eng = nc.gpsimd
                    eng.dma_start(out=outp[:, r0:r0 + w2],
                                  in_=stage[0:C, r0:r0 + w2])
    return nc


def _split_multi_waits(nc):
    """Walrus codegen allows a single sync-wait slot per instruction struct;
    hoist all but the last wait of any multi-wait instruction onto preceding
    single-wait EventSemaphore instructions on the same engine queue."""
    import concourse.mybir as mybir

    k = 0
    for bb in nc.m.functions[0].blocks:
        new = []
        changed = False
        for inst in bb.instructions:
            si = inst.sync_info
            if si is not None and si.on_wait and len(si.on_wait) > 1:
                waits = list(si.on_wait)
                for w in waits[:-1]:
                    ev = mybir.InstEventSemaphore(
                        name=f"wsplit-{k}", ins=[], outs=[])
                    k += 1
                    ev.engine = inst.engine
                    ev.sync_info = mybir.SyncInfo(on_wait=[w], on_update=[])
                    nc.inst_map[ev.name] = ev
                    new.append(ev)
                si.on_wait = [waits[-1]]
                changed = True
            new.append(inst)
        if changed:
            try:
                bb.instructions = new
            except Exception:
                bb.instructions[:] = new
    return nc


def kernel(feats, img_trans, img_scale, lidar2img):
    from concourse import bass_utils

    feats = np.ascontiguousarray(feats, dtype=np.float32)
    img_trans = np.asarray(img_trans, dtype=np.float32)
    img_scale = np.asarray(img_scale, dtype=np.float32)
    lidar2img = np.asarray(lidar2img, dtype=np.float32)
    B, N, D, H, W, C = feats.shape
    npt = B * N * D * H * W

    flatm, (NX, NY, NZ) = _host_geometry(img_trans, img_scale, lidar2img,
                                         B, N, D, H, W)
    out = np.zeros((B, NZ * C, NX, NY), np.float32)
    ib = flatm >= 0
    if not ib.any():
        return out
    uvox, slot_all = np.unique(flatm[ib], return_inverse=True)
    S = len(uvox)

    # shared padded layout: slot s owns ceil(cnt_s/8) positions on every core
    cnt = np.bincount(slot_all, minlength=S)
    m = -(-cnt // N_CORES)
    pos = np.zeros(S + 1, np.int64)
    pos[1:] = np.cumsum(m)
    M = int(pos[-1])
    nch = -(-M // P)
    Mp = nch * P
    slot_asc = np.full(Mp, -1, np.int64)
    slot_asc[:M] = np.repeat(np.arange(S), m)
    # process chunks in descending-slot order: the sparse high-slot tail
    # (many columns finishing at once) streams first and its writebacks
    # overlap the stream; the final chunks touch only the few densest
    # voxels, so the post-stream tail copies almost nothing
    slot_of_pos = slot_asc.reshape(nch, P)[::-1].reshape(-1)

    mms, lids, n_mmp, last_mm_of_bank = _build_schedule(slot_of_pos, nch, S)

    # per-voxel round-robin split of points across cores
    srt = np.argsort(slot_all, kind='stable')
    ss = slot_all[srt]
    starts = np.zeros(S, np.int64)
    starts[1:] = np.cumsum(cnt)[:-1]
    rank = np.arange(len(ss)) - starts[ss]
    core_of = rank % N_CORES
    lpos_asc = pos[ss] + rank // N_CORES
    lpos = (nch - 1 - lpos_asc // P) * P + lpos_asc % P
    feats_ib = feats.reshape(npt, C)[ib][srt].astype(BF16)

    MW = OH_W + n_mmp
    meta_np = np.zeros((P, MW), np.float32)
    meta_np[:, :OH_W] = np.arange(OH_W, dtype=np.float32)[None, :]
    meta_np[:, OH_W:] = lids
    meta_np = meta_np.astype(BF16)

    nc = _build_bass(nch, n_mmp, mms, last_mm_of_bank, S, C)
    _split_multi_waits(nc)

    in_maps = []
    for core in range(N_CORES):
        stream = np.zeros((Mp, C), BF16)
        sel = core_of == core
        stream[lpos[sel]] = feats_ib[sel]
        pts_c = np.ascontiguousarray(
            stream.reshape(nch, P, C).transpose(1, 0, 2).reshape(P, nch * C))
        in_maps.append({"pts": pts_c, "meta": meta_np})

    if bool(int(os.environ.get("BEV_TIMELINE", "0"))):
        from concourse.timeline_sim import TimelineSim
        t_ns = TimelineSim(nc).simulate()
        print(f"HW exec time: {t_ns:.0f} ns")
    res = bass_utils.run_bass_kernel_spmd(
        nc, in_maps, core_ids=list(range(N_CORES)))

    total = np.zeros((C, S), np.float64)
    for r in res.results:
        total += np.asarray(r["out"], dtype=np.float64)
    total = total.astype(np.float32)

    gsz = NZ * NX * NY
    b_u = uvox // gsz
    r_u = uvox % gsz
    z_u = r_u // (NX * NY)
    xy_u = r_u % (NX * NY)
    ov = out.reshape(B, NZ, C, NX * NY)
    ov[b_u, z_u, :, xy_u] = total.T
    return out
